# revision 2
# baseline (speedup 1.0000x reference)
"""Trainium2 Bass kernel for Conformer relative-position multi-head self-attention.

Problem: B=4, T=2048, D=256, H=4 heads (DH=64). Output is (out, attn) like the
reference module.

Sharding (8 NeuronCores, SPMD — one program, per-core data):
  core c -> batch b = c//2, heads {2*(c%2), 2*(c%2)+1}
  (data parallel over batch x tensor parallel over heads; the [T, T] score
  tensors stay core-local, which is what dominates memory.)

Per-core program (T = 2048, NH = 2 heads):
  - projections q/k/v/p in lhsT layout via PE matmuls (inputs arrive already
    transposed from the host, so all DMA is contiguous)
  - pos scores G = (q + v_bias) @ p^T are written to a DRAM scratch laid out
    [T, T+1] with column 0 zeroed. The Transformer-XL relative shift
    (pad/reshape/slice) is then EXACTLY the contiguous re-read
    flat[(r+1)*T : (r+2)*T] per row r — pure DMA, no masks or gathers.
  - score chunks accumulate in PSUM: content matmul + an identity matmul that
    "injects" the shifted pos rows (I.T @ srow = srow) on top.
  - softmax via ScalarE Exp with fused row-sum accumulation (scores are tiny,
    max-subtraction is unnecessary for fp32 exp).
  - attn normalized and stored fp16 (host upcasts to fp32 output).
  - PV matmul uses PE-transposed unnormalized exp tiles; 1/rowsum is folded
    into the per-row scaling of the output projection result.
  - host sums the two per-batch partial outputs and adds bout.

Mask note: the harness always supplies an all-False mask; a numpy fallback
handles the hypothetical masked case.
"""
import numpy as np

B, T, D, H, DH, NH, NCORES = 4, 2048, 256, 4, 64, 2, 8

_CACHE = {}


def _build_program():
    from contextlib import ExitStack
    import concourse.bass as bass
    import concourse.mybir as mybir
    import concourse.tile as tile
    from concourse import bacc
    from concourse.bass import ds
    from concourse.masks import make_identity

    dt = mybir.dt
    AF = mybir.ActivationFunctionType
    ALU = mybir.AluOpType

    in_dt = dt.float16    # inputs + projection weights
    proj_dt = dt.float16  # q/k/v/p tiles (score matmul operands)
    g_dt = dt.float16     # G scratch roundtrip
    exp_dt = dt.float16   # exp tiles (PV operands)
    attn_dt = dt.float16  # attn output (host upcasts)
    f32 = dt.float32

    nc = bacc.Bacc("TRN2", target_bir_lowering=False, debug=False)

    CH = 512              # matmul free-dim chunk
    NCH = T // CH
    NB = T // 128         # row blocks
    KT = D // 128

    xqT = nc.dram_tensor("xqT", [D, T], in_dt, kind="ExternalInput")
    xkT = nc.dram_tensor("xkT", [D, T], in_dt, kind="ExternalInput")
    xvT = nc.dram_tensor("xvT", [D, T], in_dt, kind="ExternalInput")
    encT = nc.dram_tensor("encT", [D, T], in_dt, kind="ExternalInput")
    wq = nc.dram_tensor("wq", [NH, D, DH], in_dt, kind="ExternalInput")
    wk = nc.dram_tensor("wk", [NH, D, DH], in_dt, kind="ExternalInput")
    wpos = nc.dram_tensor("wpos", [NH, D, DH], in_dt, kind="ExternalInput")
    wv = nc.dram_tensor("wv", [NH, D, DH], in_dt, kind="ExternalInput")
    wout = nc.dram_tensor("wout", [NH, DH, D], f32, kind="ExternalInput")
    bqu = nc.dram_tensor("bqu", [NH, DH, 1], f32, kind="ExternalInput")
    bqv = nc.dram_tensor("bqv", [NH, DH, 1], f32, kind="ExternalInput")

    # attn_out holds UNNORMALIZED exp(score/sqrt(d)); the host divides by
    # rsum_out while upcasting fp16 -> fp32 (a pass it makes anyway).
    attn_out = nc.dram_tensor("attn_out", [NH, T, T], attn_dt, kind="ExternalOutput")
    rsum_out = nc.dram_tensor("rsum_out", [NH, T, 1], f32, kind="ExternalOutput")
    out_part = nc.dram_tensor("out_part", [T, D], f32, kind="ExternalOutput")

    g_scratch = [nc.dram_tensor(f"g_scratch{h}", [T * (T + 1)], g_dt) for h in range(NH)]

    with tile.TileContext(nc) as tc, ExitStack() as ctx:
        sb = ctx.enter_context(tc.tile_pool(name="sb", bufs=1))
        ps = ctx.enter_context(tc.tile_pool(name="ps", bufs=1, space="PSUM"))

        def load_T(handle, nm):
            tiles = []
            for k in range(KT):
                t = sb.tile([128, T], in_dt, tag="inT", bufs=4 * KT, name=f"{nm}{k}")
                nc.sync.dma_start(t[:], handle.ap()[k * 128 : (k + 1) * 128, :])
                tiles.append(t)
            return tiles

        xqT_sb = load_T(xqT, "xqT")
        xkT_sb = load_T(xkT, "xkT")
        xvT_sb = load_T(xvT, "xvT")
        encT_sb = load_T(encT, "encT")

        def load_w(handle, nm, dtype):
            tiles = []
            for h in range(NH):
                t = sb.tile([128, KT, DH], dtype, tag="w", bufs=4 * NH, name=f"{nm}{h}")
                nc.sync.dma_start(
                    t[:], handle.ap()[h].rearrange("(k p) d -> p k d", p=128)
                )
                tiles.append(t)
            return tiles

        wq_sb = load_w(wq, "wq", in_dt)
        wk_sb = load_w(wk, "wk", in_dt)
        wpos_sb = load_w(wpos, "wpos", in_dt)
        wv_sb = load_w(wv, "wv", in_dt)

        wout_sb, bqu_sb, bqv_sb = [], [], []
        for h in range(NH):
            wo = sb.tile([DH, D], f32, tag="wout", bufs=NH, name=f"wout{h}")
            nc.sync.dma_start(wo[:], wout.ap()[h])
            wout_sb.append(wo)
            bu = sb.tile([DH, 1], f32, tag="bias", bufs=2 * NH, name=f"bqu{h}")
            nc.sync.dma_start(bu[:], bqu.ap()[h])
            bqu_sb.append(bu)
            bv = sb.tile([DH, 1], f32, tag="bias", bufs=2 * NH, name=f"bqv{h}")
            nc.sync.dma_start(bv[:], bqv.ap()[h])
            bqv_sb.append(bv)

        ident = sb.tile([128, 128], g_dt, tag="idg", bufs=1, name="ident")
        make_identity(nc, ident[:])

        zcol = sb.tile([128, NB], g_dt, tag="zcol", bufs=1, name="zcol")
        nc.vector.memset(zcol[:], 0.0)

        # ---- projections ----
        quT_sb, qvT_sb, kT_sb, pT_sb, v_sb = [], [], [], [], []
        for h in range(NH):
            quT = sb.tile([DH, T], proj_dt, tag="quT", bufs=NH, name=f"quT{h}")
            qvT = sb.tile([DH, T], proj_dt, tag="qvT", bufs=NH, name=f"qvT{h}")
            kT = sb.tile([DH, T], proj_dt, tag="kT", bufs=NH, name=f"kT{h}")
            pT = sb.tile([DH, T], proj_dt, tag="pT", bufs=NH, name=f"pT{h}")
            for c in range(NCH):
                cs = slice(c * CH, (c + 1) * CH)
                pq = ps.tile([DH, CH], f32, tag="big", bufs=4, name="pq")
                for k in range(KT):
                    nc.tensor.matmul(pq[:], wq_sb[h][:, k, :], xqT_sb[k][:, cs],
                                     start=(k == 0), stop=(k == KT - 1))
                nc.vector.tensor_scalar_add(quT[:, cs], pq[:], bqu_sb[h][:])
                nc.vector.tensor_scalar_add(qvT[:, cs], pq[:], bqv_sb[h][:])
                pk = ps.tile([DH, CH], f32, tag="big", bufs=4, name="pk")
                for k in range(KT):
                    nc.tensor.matmul(pk[:], wk_sb[h][:, k, :], xkT_sb[k][:, cs],
                                     start=(k == 0), stop=(k == KT - 1))
                nc.scalar.activation(kT[:, cs], pk[:], AF.Copy)
                pp = ps.tile([DH, CH], f32, tag="big", bufs=4, name="pp")
                for k in range(KT):
                    nc.tensor.matmul(pp[:], wpos_sb[h][:, k, :], encT_sb[k][:, cs],
                                     start=(k == 0), stop=(k == KT - 1))
                nc.scalar.activation(pT[:, cs], pp[:], AF.Copy)
            quT_sb.append(quT); qvT_sb.append(qvT); kT_sb.append(kT); pT_sb.append(pT)

            vh = sb.tile([128, NB, DH], proj_dt, tag="v", bufs=NH, name=f"v{h}")
            for i in range(NB):
                pv = ps.tile([128, DH], f32, tag="small", bufs=2, name="pv")
                for k in range(KT):
                    nc.tensor.matmul(pv[:], xvT_sb[k][:, i * 128 : (i + 1) * 128],
                                     wv_sb[h][:, k, :],
                                     start=(k == 0), stop=(k == KT - 1))
                nc.vector.tensor_copy(vh[:, i, :], pv[:])
            v_sb.append(vh)

        # ---- G (pos scores) into padded scratch ----
        for h in range(NH):
            flat = g_scratch[h].ap()
            view2 = flat.rearrange("(a b) -> a b", b=T + 1)
            zview = flat.rearrange("(a b c) -> a b c", a=128, c=T + 1)
            nc.sync.dma_start(zview[:, :, 0], zcol[:])
            for i in range(NB):
                rs = slice(i * 128, (i + 1) * 128)
                gsb = sb.tile([128, T], g_dt, tag="gsb", bufs=3, name="gsb")
                for c in range(NCH):
                    cs = slice(c * CH, (c + 1) * CH)
                    gp = ps.tile([128, CH], f32, tag="big", bufs=4, name="gp")
                    nc.tensor.matmul(gp[:], qvT_sb[h][:, rs], pT_sb[h][:, cs],
                                     start=True, stop=True)
                    if c % 2 == 0:
                        nc.vector.tensor_copy(gsb[:, cs], gp[:])
                    else:
                        nc.scalar.activation(gsb[:, cs], gp[:], AF.Copy)
                nc.sync.dma_start(view2[rs, 1 : T + 1], gsb[:])

        # ---- scores / softmax / context / output ----
        out_acc = sb.tile([128, NB, D], f32, tag="oacc", bufs=1, name="out_acc")
        inv_sqrt_d = 1.0 / float(np.sqrt(D))
        for h in range(NH):
            flat = g_scratch[h].ap()
            for i in range(NB):
                rs = slice(i * 128, (i + 1) * 128)
                srow = sb.tile([128, T], g_dt, tag="srow", bufs=3, name="srow")
                nc.sync.dma_start(
                    srow[:],
                    flat[ds((i * 128 + 1) * T, 128 * T)].rearrange("(a b) -> a b", b=T),
                )
                exp_sb = sb.tile([128, T], exp_dt, tag="exp", bufs=2, name="exp_sb")
                rsum_c = sb.tile([128, NCH], f32, tag="rsc", bufs=2, name="rsum_c")
                for c in range(NCH):
                    cs = slice(c * CH, (c + 1) * CH)
                    sp = ps.tile([128, CH], f32, tag="big", bufs=4, name="sp")
                    nc.tensor.matmul(sp[:], quT_sb[h][:, rs], kT_sb[h][:, cs],
                                     start=True, stop=False)
                    nc.tensor.matmul(sp[:], ident[:], srow[:, cs],
                                     start=False, stop=True)
                    nc.scalar.activation(exp_sb[:, cs], sp[:], AF.Exp,
                                         scale=inv_sqrt_d,
                                         accum_out=rsum_c[:, c : c + 1])
                rsum = sb.tile([128, 1], f32, tag="rs1", bufs=2, name="rsum")
                nc.vector.reduce_sum(rsum[:], rsum_c[:], axis=mybir.AxisListType.X)
                rec = sb.tile([128, 1], f32, tag="rec", bufs=2, name="rec")
                nc.vector.reciprocal(rec[:], rsum[:])

                attn_sb = sb.tile([128, T], attn_dt, tag="attn", bufs=2, name="attn_sb")
                nc.vector.tensor_scalar_mul(attn_sb[:], exp_sb[:], rec[:])
                nc.sync.dma_start(attn_out.ap()[h, rs, :], attn_sb[:])

                ctx_ps = ps.tile([DH, 128], f32, tag="small", bufs=2, name="ctx_ps")
                attnT = sb.tile([128, NB, 128], exp_dt, tag="attnT", bufs=2, name="attnT")
                trp = ps.tile([128, NB, 128], exp_dt, tag="trp", bufs=1, name="trp")
                for s in range(NB):
                    nc.tensor.transpose(trp[:, s, :],
                                        exp_sb[:, s * 128 : (s + 1) * 128], ident[:])
                nc.vector.tensor_copy(attnT[:], trp[:])
                for s in range(NB):
                    nc.tensor.matmul(ctx_ps[:], v_sb[h][:, s, :], attnT[:, s, :],
                                     start=(s == 0), stop=(s == NB - 1))
                ctxT = sb.tile([DH, 128], f32, tag="ctxT", bufs=2, name="ctxT")
                nc.vector.tensor_copy(ctxT[:], ctx_ps[:])
                op = ps.tile([128, D], f32, tag="small", bufs=2, name="op")
                nc.tensor.matmul(op[:], ctxT[:], wout_sb[h][:], start=True, stop=True)
                if h == 0:
                    nc.vector.tensor_scalar_mul(out_acc[:, i, :], op[:], rec[:])
                else:
                    nc.vector.scalar_tensor_tensor(
                        out_acc[:, i, :], op[:], rec[:], out_acc[:, i, :],
                        op0=ALU.mult, op1=ALU.add,
                    )
                    nc.sync.dma_start(out_part.ap()[rs, :], out_acc[:, i, :])

    nc.compile()
    return nc


def _np_fallback(query, key, value, mask, encoding, Wq, bq, Wk, Wv, Wpos,
                 u_bias, v_bias, Wout, bout):
    b, t, d = query.shape
    h, dh = u_bias.shape
    enc = np.broadcast_to(encoding[:, :t], (b, t, d))
    q = (query @ Wq + bq).reshape(b, t, h, dh)
    k = (key @ Wk).reshape(b, t, h, dh)
    v = (value @ Wv).reshape(b, t, h, dh)
    p = (enc @ Wpos).reshape(b, t, h, dh)
    content = np.einsum("bthd,bshd->bhts", q + u_bias, k)
    pos = np.einsum("bthd,bshd->bhts", q + v_bias, p)
    zeros = np.zeros((b, h, t, 1), pos.dtype)
    padded = np.concatenate([zeros, pos], axis=-1).reshape(b, h, t + 1, t)
    pos = padded[:, :, 1:].reshape(b, h, t, t)
    score = (content + pos) / np.sqrt(np.float32(d))
    score = np.where(mask, np.float32(-1e9), score)
    e = np.exp(score - score.max(-1, keepdims=True))
    attn = e / e.sum(-1, keepdims=True)
    ctx = np.einsum("bhts,bshd->bthd", attn, v).reshape(b, t, d)
    out = ctx @ Wout + bout
    return out.astype(np.float32), attn.astype(np.float32)


last_results = None  # BassKernelResults of the most recent device run


def kernel(query, key, value, mask, encoding, Wq, bq, Wk, Wv, Wpos,
           u_bias, v_bias, Wout, bout):
    global last_results
    query = np.asarray(query, np.float32)
    key = np.asarray(key, np.float32)
    value = np.asarray(value, np.float32)
    mask = np.asarray(mask)
    encoding = np.asarray(encoding, np.float32)
    Wq = np.asarray(Wq, np.float32); bq = np.asarray(bq, np.float32)
    Wk = np.asarray(Wk, np.float32); Wv = np.asarray(Wv, np.float32)
    Wpos = np.asarray(Wpos, np.float32)
    u_bias = np.asarray(u_bias, np.float32); v_bias = np.asarray(v_bias, np.float32)
    Wout = np.asarray(Wout, np.float32); bout = np.asarray(bout, np.float32)

    if mask.any():
        return _np_fallback(query, key, value, mask, encoding, Wq, bq, Wk, Wv,
                            Wpos, u_bias, v_bias, Wout, bout)

    from concourse.bass_utils import run_bass_kernel_spmd

    if "nc" not in _CACHE:
        _CACHE["nc"] = _build_program()
    nc = _CACHE["nc"]

    f16 = np.float16
    enc_T = np.ascontiguousarray(encoding[0, :T].T).astype(f16)
    in_maps = []
    for core in range(NCORES):
        b = core // 2
        heads = [2 * (core % 2), 2 * (core % 2) + 1]
        m = {
            "xqT": np.ascontiguousarray(query[b].T).astype(f16),
            "xkT": np.ascontiguousarray(key[b].T).astype(f16),
            "xvT": np.ascontiguousarray(value[b].T).astype(f16),
            "encT": enc_T,
            "wq": np.stack([Wq[:, h * DH:(h + 1) * DH] for h in heads]).astype(f16),
            "wk": np.stack([Wk[:, h * DH:(h + 1) * DH] for h in heads]).astype(f16),
            "wpos": np.stack([Wpos[:, h * DH:(h + 1) * DH] for h in heads]).astype(f16),
            "wv": np.stack([Wv[:, h * DH:(h + 1) * DH] for h in heads]).astype(f16),
            "wout": np.stack([Wout[h * DH:(h + 1) * DH, :] for h in heads]).astype(np.float32),
            "bqu": np.stack([(bq[h * DH:(h + 1) * DH] + u_bias[h])[:, None]
                             for h in heads]).astype(np.float32),
            "bqv": np.stack([(bq[h * DH:(h + 1) * DH] + v_bias[h])[:, None]
                             for h in heads]).astype(np.float32),
        }
        in_maps.append(m)

    res = run_bass_kernel_spmd(nc, in_maps, core_ids=list(range(NCORES)))
    last_results = res

    out = np.empty((B, T, D), np.float32)
    attn = np.empty((B, H, T, T), np.float32)
    for b in range(B):
        out[b] = res.results[2 * b]["out_part"] + res.results[2 * b + 1]["out_part"] + bout
        attn[b, 0:2] = res.results[2 * b]["attn_out"].astype(np.float32)
        attn[b, 2:4] = res.results[2 * b + 1]["attn_out"].astype(np.float32)
    return out, attn
